# revision 1
# baseline (speedup 1.0000x reference)
"""MHA (RoPE + causal softmax attention + out-proj) on 8 NeuronCores.

Sharding: DP4 x TP2. Core c: batch b = c % 4, head-group g = c // 4
(8 heads per core). Each core computes a transposed partial output
outT = (y_local @ w_o_slice^T)^T in [D, L]; host sums the two head-group
partials per batch (fp16) and transposes back.

All matmuls fp16 x fp16 -> fp32 PSUM (fp16 runs at the same PE rate as
bf16 in the cost model but with 10-bit mantissas). Layout strategy:
  Phase A: qkv natural layout [L, comps] via out = xT_tile.T @ w_chunk.
           Chunk 0 runs d-outer over 7 PSUM accumulators so matmuls can
           chase the startup DMAs (issue order is tuned; chunk-0 weights
           are loaded in per-d slices interleaved with the x tiles).
           RoPE applied with strided free-dim APs straight out of PSUM;
           rotated q/k staged to DRAM scratch (fp16) for the transposed
           reload; v copied directly into SBUF group tiles (no DRAM).
           The RoPE working pools close after chunk 4 so attention for
           (heads 0-3, q-chunk 0) interleaves into chunk 5 (v47), hiding
           its exp-bound work under the last GEMM chunk.
  Phase B: per head, q/k loaded back transposed ([comps, L]) via DMA xbar
           transpose; scores computed transposed (k on partitions) so the
           attn weights are ready as the moving operand of attn@V.
           exp on ScalarE with the 1/sqrt(HD) scale fused. Causal handled
           exactly at 128-col granularity: fully-masked k-tiles skipped,
           diagonal tiles column-trimmed, one [128,128] triangular mask.
           Softmax denominator accumulated on DVE (copy+adds over the
           fp16 attn tiles) with a single all-ones matmul per (h, qc)
           for the partition reduction (keeps TensorE free). Denominator
           reduce + normalize are deferred one block so PE never waits
           on the DVE add-chain.
  Phase C: out-proj outT[e, q] = sum_d w_oT[d, e] * yT[d, q]. As soon as
           a q-chunk's 8 heads are normalized, its 16 out-proj tiles are
           queued and fed into later attention blocks as PE filler, so
           the Activation engine (exp) never starves the PE; heads 0-3
           come first in the contraction so the tail does not wait on
           the last head's softmax chain.
"""

import contextlib

import numpy as np

import concourse.tile as tile
import concourse.mybir as mybir
from concourse import bacc
from concourse.bass_utils import run_bass_kernel_spmd

F16 = np.float16
F32 = mybir.dt.float32
FP16 = mybir.dt.float16

B, L, D, H, HD = 4, 2048, 2048, 16, 128
NH = 8                      # heads per core
DL = NH * HD                # 1024 local head dims
ROPE_BASE = 10000.0
ALPHA = float(HD) ** -0.5

LT = L // 128               # 16 L-tiles
DT = D // 128               # 16 D(contract)-tiles
NCH = 6                     # qkv chunks of 512 comps: q03,k03,v03,q47,k47,v47
QC = L // 512               # 4 q-chunks of 512
KT = L // 128               # 16 k-tiles


def _chunk_kind(c):
    # chunk order: q(heads0-3), k(0-3), v(0-3), q(4-7), k(4-7), v(4-7)
    return ("q", "k", "v")[c % 3], c // 3


def build_program(la=5, scb=3):
    nc = bacc.Bacc("TRN2", target_bir_lowering=False, debug=False, num_devices=8)

    xT = nc.dram_tensor("xT", [D, L], FP16, kind="ExternalInput").ap()
    wqkvT = nc.dram_tensor("wqkvT", [D, 3 * DL], FP16, kind="ExternalInput").ap()
    woT = nc.dram_tensor("woT", [DL, L], FP16, kind="ExternalInput").ap()
    chalf = nc.dram_tensor("chalf", [L, 256], FP16, kind="ExternalInput").ap()
    shalf = nc.dram_tensor("shalf", [L, 256], FP16, kind="ExternalInput").ap()
    trimask = nc.dram_tensor("trimask", [128, 128], FP16, kind="ExternalInput").ap()
    outT = nc.dram_tensor("outT", [D, L], FP16, kind="ExternalOutput").ap()

    # DRAM staging for rotated q/k (natural layout) only; v stays in SBUF
    qrot = nc.dram_tensor("qrot", [L, DL], FP16, kind="Internal").ap()
    krot = nc.dram_tensor("krot", [L, DL], FP16, kind="Internal").ap()

    with tile.TileContext(nc) as tc, contextlib.ExitStack() as es:
        pr = es.enter_context(tc.tile_pool(name="pR", bufs=1, side="right"))
        pqk03 = es.enter_context(
            tc.tile_pool(name="pQK03", bufs=1, side="right"))
        qkts03 = []
        qkts47 = []
        yts = [None] * NH
        vts = []
        wos = []

        # ---------------- attention block emitter ----------------
        def attn_head_qc(pools, h, qc, pe_denom=False, dpool=None,
                         filler=None):
            """Emit attention for (head h, q-chunk qc); returns a finale
            closure (denominator reduce + normalize) the caller defers
            so PE never blocks on the DVE add-chain."""
            pss, pba, psy, psd, pbr = pools
            g, hl = h // 4, h % 4
            qt, kt = (qkts03 + qkts47)[h]
            nkt = 4 * qc + 4
            ypsum = psy.tile([128, 512], F32, name="ypsum", tag="yp")
            dacc = None
            if pe_denom:
                # accumulated on PE during the block; borrow a phase-C bank
                # (psD would WAR-deadlock against the deferred finales)
                dpsum = dpool()
            else:
                dpsum = psd.tile([128, 512], F32, name="dpsum", tag="dp")
                dacc = pbr.tile([128, 512], FP16, name="dacc", tag="dacc")
            ats = {}

            def emit_score(j):
                m = j - 4 * qc  # >= 0 on the diagonal block
                c0 = max(m, 0) * 128  # first valid within-chunk col
                sc = pss.tile([128, 512], F32, name="sc", tag="sc")
                nc.tensor.matmul(
                    sc[:, c0:], kt[:, j * 128:(j + 1) * 128],
                    qt[:, qc * 512 + c0:(qc + 1) * 512],
                    start=True, stop=True)
                at = pba.tile([128, 512], FP16, name="at", tag="at")
                nc.scalar.activation(
                    out=at[:, c0:], in_=sc[:, c0:],
                    func=mybir.ActivationFunctionType.Exp,
                    scale=ALPHA)
                if m >= 0:
                    nc.vector.tensor_mul(
                        at[:, c0:c0 + 128], at[:, c0:c0 + 128], mt)
                ats[j] = at

            def emit_dadd_at(j, at):
                m = j - 4 * qc
                c0 = max(m, 0) * 128
                if pe_denom:
                    nc.tensor.matmul(
                        dpsum[:, c0:], ones128, at[:, c0:],
                        start=(j == 0), stop=(j == nkt - 1),
                        skip_group_check=True)
                elif j == 0:
                    nc.vector.tensor_copy(out=dacc, in_=at)
                else:
                    nc.vector.tensor_add(
                        dacc[:, c0:], dacc[:, c0:], at[:, c0:])

            n_off = 4 * qc
            next_emit = 0

            def emit_upto(n):
                nonlocal next_emit
                while next_emit < n:
                    emit_score(next_emit)
                    next_emit += 1

            emit_upto(min(la, nkt))
            if filler is not None:
                # PE work between the first scores and the first attn@V
                # consume hides the exp latency at block start
                filler()
            # off-diagonal k-tiles: full-width attn@V
            for j in range(n_off):
                emit_upto(min(j + 1 + la, nkt))
                at = ats.pop(j)
                nc.tensor.matmul(
                    ypsum, vts[g][:, j, hl * 128:(hl + 1) * 128], at,
                    start=(j == 0), stop=False)
                emit_dadd_at(j, at)
                if filler is not None and j % 2 == 1:
                    filler()

            # diagonal block: make sure all 4 at tiles exist first
            emit_upto(nkt)
            if filler is not None:
                filler()
                filler()
            d_ats = [ats.pop(4 * qc + m) for m in range(4)]
            for m in range(4):
                emit_dadd_at(4 * qc + m, d_ats[m])
            for mq in range(4):
                for m in range(mq + 1):
                    nc.tensor.matmul(
                        ypsum[:, mq * 128:(mq + 1) * 128],
                        vts[g][:, 4 * qc + m, hl * 128:(hl + 1) * 128],
                        d_ats[m][:, mq * 128:(mq + 1) * 128],
                        start=(qc == 0 and m == 0), stop=(m == mq),
                        skip_group_check=True)

            def finale():
                if not pe_denom:
                    nc.tensor.matmul(dpsum, ones128, dacc,
                                     start=True, stop=True)
                rbs = pbr.tile([128, 512], FP16, name="rbs", tag="rbs")
                with nc.allow_low_precision("softmax recip fp16"):
                    nc.vector.reciprocal(out=rbs, in_=dpsum)
                nc.vector.tensor_mul(
                    yts[h][:, qc * 512:(qc + 1) * 512], ypsum, rbs)

            return finale

        with tc.tile_pool(name="pAx", bufs=1) as pax, \
             tc.tile_pool(name="pAw", bufs=3) as paw:
            es_rope = contextlib.ExitStack()
            pcs = es_rope.enter_context(tc.tile_pool(name="pCs", bufs=1))
            pat = es_rope.enter_context(tc.tile_pool(name="pAt", bufs=2))
            pao = es_rope.enter_context(tc.tile_pool(name="pAo", bufs=3))
            # ------- resident left-side tiles + startup DMA order -------
            # DMA issue order is everything here: interleave per-d weight
            # slices with the x tiles the first 7 psum groups need, so
            # the d-outer matmuls start ~3.3us in and chase the queue.
            wch0 = paw.tile([128, DT, 512], FP16, name="wch", tag="wch")
            xts = []
            c_sb = pcs.tile([128, LT, 256], FP16, name="c_sb", tag="c_sb")
            s_sb = pcs.tile([128, LT, 256], FP16, name="s_sb", tag="s_sb")
            for d in range(7):
                nc.sync.dma_start(
                    out=wch0[:, d, :],
                    in_=wqkvT[d * 128:(d + 1) * 128, 0:512])
                xt = pax.tile([128, L], FP16, name=f"xt{d}", tag=f"xt{d}")
                nc.sync.dma_start(out=xt, in_=xT[d * 128:(d + 1) * 128, :])
                xts.append(xt)
            for d in range(7, DT):
                nc.sync.dma_start(
                    out=wch0[:, d, :],
                    in_=wqkvT[d * 128:(d + 1) * 128, 0:512])
            nc.sync.dma_start(
                out=c_sb[:, 0:8, :],
                in_=chalf[0:1024, :].rearrange("(i p) g -> p i g", p=128))
            nc.sync.dma_start(
                out=s_sb[:, 0:8, :],
                in_=shalf[0:1024, :].rearrange("(i p) g -> p i g", p=128))
            for d in range(7, DT):
                xt = pax.tile([128, L], FP16, name=f"xt{d}", tag=f"xt{d}")
                nc.sync.dma_start(out=xt, in_=xT[d * 128:(d + 1) * 128, :])
                xts.append(xt)
            nc.sync.dma_start(
                out=c_sb[:, 8:, :],
                in_=chalf[1024:, :].rearrange("(i p) g -> p i g", p=128))
            nc.sync.dma_start(
                out=s_sb[:, 8:, :],
                in_=shalf[1024:, :].rearrange("(i p) g -> p i g", p=128))

            mt = pr.tile([128, 128], FP16, name="trimask_sb", tag="mask")
            nc.sync.dma_start(out=mt, in_=trimask)
            ones128 = pr.tile([128, 128], FP16, name="ones128", tag="oc")
            nc.vector.memset(ones128, 1.0)

            for g in range(2):
                vt = pr.tile([128, LT, 512], FP16, name=f"vt{g}",
                             tag=f"vt{g}")
                vts.append(vt)

            # ---------------- Phase A: QKV + RoPE ----------------
            def rope_evac(pnat, i, kind, grp):
                """Evacuate one [128,512] qkv psum tile."""
                if kind == "v":
                    nc.scalar.copy(out=vts[grp][:, i, :], in_=pnat)
                    return
                x1 = pnat[:, 0::2]
                x2 = pnat[:, 1::2]
                ct = c_sb[:, i, :]
                st = s_sb[:, i, :]
                t1 = pat.tile([128, 256], F32, name="t1", tag="t1")
                nc.vector.tensor_mul(t1, x1, ct)
                t2 = pat.tile([128, 256], F32, name="t2", tag="t2")
                nc.vector.tensor_mul(t2, x2, st)
                t3 = pat.tile([128, 256], F32, name="t3", tag="t3")
                nc.vector.tensor_mul(t3, x2, ct)
                t4 = pat.tile([128, 256], F32, name="t4", tag="t4")
                nc.vector.tensor_mul(t4, x1, st)
                ro = pao.tile([128, 512], FP16, name="ro", tag="ro")
                nc.vector.tensor_sub(ro[:, 0::2], t1, t2)
                nc.vector.tensor_add(ro[:, 1::2], t3, t4)
                dst = qrot if kind == "q" else krot
                nc.sync.dma_start(
                    out=dst[i * 128:(i + 1) * 128,
                            grp * 512:(grp + 1) * 512],
                    in_=ro)

            # PSUM: psAV (2 banks, for the v47 chunk that overlaps phase B
            # pools) opens BEFORE psA6 so the pools can close in LIFO order
            # with no transition barrier at chunk 5.
            psav = es.enter_context(
                tc.tile_pool(name="psAV", bufs=1, space="PSUM"))

            def pv_tile(slot):
                return psav.tile([128, 512], F32, name="pv",
                                 tag=f"pv{slot}")

            avn = [0]

            def av_tile():
                t = pv_tile(avn[0] % 2)
                avn[0] += 1
                return t

            with tc.tile_pool(name="psA6", bufs=1, space="PSUM") as psa6:
                def pn_tile(slot):
                    return psa6.tile([128, 512], F32, name="pn",
                                     tag=f"pn{slot}")

                def c0_tile(i):
                    # chunk 0 borrows the psAV banks too: 8-deep rotation
                    s = i % 8
                    return pn_tile(s) if s < 6 else pv_tile(s - 6)

                # chunk 0 (q03) first part: d-outer over 7 psum banks so
                # the matmuls chase the startup DMAs (bank 8 stays free
                # for i=7 so it needn't wait on any evacuation).
                pns = [c0_tile(ii) for ii in range(7)]
                for d in range(DT):
                    for ii in range(7):
                        nc.tensor.matmul(
                            pns[ii],
                            xts[d][:, ii * 128:(ii + 1) * 128],
                            wch0[:, d, :],
                            start=(d == 0), stop=(d == DT - 1))
                for ii in range(7):
                    rope_evac(pns[ii], ii, "q", 0)
                # chunk 0 rest: data resident, i-outer so the RoPE
                # evacuations drain while the matmuls continue.
                for i in range(7, LT):
                    pnat = c0_tile(i)
                    for d in range(DT):
                        nc.tensor.matmul(
                            pnat,
                            xts[d][:, i * 128:(i + 1) * 128],
                            wch0[:, d, :],
                            start=(d == 0), stop=(d == DT - 1))
                    rope_evac(pnat, i, "q", 0)

                # chunks 1-4: i-outer, d-inner (weights triple-buffered),
                # cycling through the 6 psA6 banks.
                for c in range(1, NCH - 1):
                    kind, grp = _chunk_kind(c)
                    wch = paw.tile([128, DT, 512], FP16, name="wch",
                                   tag="wch")
                    nc.sync.dma_start(
                        out=wch,
                        in_=wqkvT[:, c * 512:(c + 1) * 512].rearrange(
                            "(d p) e -> p d e", p=128))
                    for i in range(LT):
                        pnat = pn_tile(i % 6)
                        for d in range(DT):
                            nc.tensor.matmul(
                                pnat,
                                xts[d][:, i * 128:(i + 1) * 128],
                                wch[:, d, :],
                                start=(d == 0), stop=(d == DT - 1))
                        rope_evac(pnat, i, kind, grp)
                    if c == 1:
                        # rotated q/k for heads 0-3 are now staged in
                        # DRAM: issue their transposed reloads early so
                        # they drain during the rest of phase A.
                        for h in range(4):
                            qt = pqk03.tile([128, L], FP16, name="qt03",
                                            tag=f"qt{h}", bufs=1)
                            nc.sync.dma_start_transpose(
                                out=qt,
                                in_=qrot[:, h * 128:(h + 1) * 128])
                            kt = pqk03.tile([128, L], FP16, name="kt03",
                                            tag=f"kt{h}", bufs=1)
                            nc.sync.dma_start_transpose(
                                out=kt,
                                in_=krot[:, h * 128:(h + 1) * 128])
                            qkts03.append((qt, kt))

            # RoPE pools (cos/sin, temps, psA8) close here; open the
            # attention pools that must outlive phase A, then emit chunk 5
            # (v47, no RoPE) with (heads 0-3, qc 0) attention interleaved.
            es_rope.close()
            pba = es.enter_context(
                tc.tile_pool(name="pBa", bufs=10, side="right"))
            pbr = es.enter_context(
                tc.tile_pool(name="pBr", bufs=2, side="right"))
            pyts = es.enter_context(
                tc.tile_pool(name="pYts", bufs=1, side="right"))
            pss = es.enter_context(
                tc.tile_pool(name="psS", bufs=scb, space="PSUM"))
            psy = es.enter_context(
                tc.tile_pool(name="psY", bufs=2, space="PSUM"))
            psd = es.enter_context(
                tc.tile_pool(name="psD", bufs=1, space="PSUM"))
            for h in range(4):
                yts[h] = pyts.tile([128, L], FP16, name=f"yt{h}",
                                   tag=f"yt{h}")
            bpools = (pss, pba, psy, psd, pbr)
            pending = None
            c = NCH - 1
            kind, grp = _chunk_kind(c)
            wch = paw.tile([128, DT, 512], FP16, name="wch", tag="wch")
            nc.sync.dma_start(
                out=wch,
                in_=wqkvT[:, c * 512:(c + 1) * 512].rearrange(
                    "(d p) e -> p d e", p=128))
            inject = {4: (0, 0), 8: (0, 1), 12: (0, 2), 15: (0, 3)}
            for i in range(LT):
                pnat = pv_tile(i % 2)
                for d in range(DT):
                    nc.tensor.matmul(
                        pnat,
                        xts[d][:, i * 128:(i + 1) * 128],
                        wch[:, d, :],
                        start=(d == 0), stop=(d == DT - 1))
                rope_evac(pnat, i, kind, grp)
                if i in inject:
                    qci, hi = inject[i]
                    fin = attn_head_qc(bpools, hi, qci, dpool=av_tile)
                    if pending is not None:
                        pending()
                    pending = fin

        # ---------------- Phase B rest + Phase C ----------------
        pqkb = es.enter_context(
            tc.tile_pool(name="pQK47", bufs=1, side="right"))
        pcw = es.enter_context(tc.tile_pool(name="pCw", bufs=1))
        pco = es.enter_context(tc.tile_pool(name="pCo", bufs=4))
        for h in range(4, NH):
            qt = pqkb.tile([128, L], FP16, name="qt47", tag=f"qt{h}")
            nc.sync.dma_start_transpose(
                out=qt, in_=qrot[:, h * 128:(h + 1) * 128])
            kt = pqkb.tile([128, L], FP16, name="kt47", tag=f"kt{h}")
            nc.sync.dma_start_transpose(
                out=kt, in_=krot[:, h * 128:(h + 1) * 128])
            qkts47.append((qt, kt))
        for h in range(4, NH):
            yts[h] = pqkb.tile([128, L], FP16, name=f"yt{h}", tag=f"yt{h}")
        for dd in range(NH):
            wo = pcw.tile([128, L], FP16, name=f"wo{dd}", tag=f"wo{dd}")
            nc.sync.dma_start(out=wo, in_=woT[dd * 128:(dd + 1) * 128, :])
            wos.append(wo)

        # remaining blocks: interleave qc1 heads 0-3 (their q/k are
        # resident) with qc0 heads 4-7 so each block waits at most one
        # in-flight transposed reload.
        seq = [(1, 0), (1, 1), (0, 4), (0, 5), (0, 6), (0, 7),
               (1, 2), (1, 3), (1, 4), (1, 5), (1, 6), (1, 7)]
        seq += [(qc, h) for qc in (2, 3) for h in range(NH)]

        # phase C borrows the psAV banks (freed after chunk 5); pools on
        # the PSUM side must close LIFO so psAV simply stays open.
        def c_tile(qc, e):
            op = av_tile()
            for dd in range(NH):
                nc.tensor.matmul(
                    op,
                    wos[dd][:, e * 128:(e + 1) * 128],
                    yts[dd][:, qc * 512:(qc + 1) * 512],
                    start=(dd == 0), stop=(dd == NH - 1))
            ot = pco.tile([128, 512], FP16, name="ot", tag="ot")
            nc.scalar.copy(out=ot, in_=op)
            nc.sync.dma_start(
                out=outT[e * 128:(e + 1) * 128,
                         qc * 512:(qc + 1) * 512],
                in_=ot)

        cwork = []
        quota = [0]
        since_refill = [0]

        def filler():
            if cwork and quota[0] > 0:
                quota[0] -= 1
                cwork.pop(0)()

        emitted_fin = {0: 3, 1: 0, 2: 0, 3: 0}  # h0-h2 flushed in A
        for k, (qc, h) in enumerate(seq):
            since_refill[0] += 1
            bl = max(1, 8 - since_refill[0])
            quota[0] = (len(cwork) + bl - 1) // bl if cwork else 0
            last = k == len(seq) - 1
            fin = attn_head_qc(bpools, h, qc, pe_denom=last,
                               dpool=av_tile, filler=filler)
            if pending is not None:
                pending()
                fq = seq[k - 1] if k > 0 else (0, 3)
                emitted_fin[fq[0]] += 1
                if emitted_fin[fq[0]] == NH and fq[0] < QC - 1:
                    cwork.extend(
                        (lambda qq, ee: lambda: c_tile(qq, ee))(
                            fq[0], e) for e in range(DT))
                    since_refill[0] = 0
            if last:
                # all-PE denominator: the finale is cheap, flush it
                # now so phase C can be emitted last
                fin()
                pending = None
            else:
                pending = fin
        quota[0] = len(cwork)
        while cwork:
            filler()
        for e in range(DT):
            c_tile(QC - 1, e)
    nc.compile()
    return nc


_NC_CACHE = None


def _get_program():
    global _NC_CACHE
    if _NC_CACHE is None:
        _NC_CACHE = build_program()
    return _NC_CACHE


def _host_inputs(x, w_qkv, w_o):
    inv = 1.0 / (ROPE_BASE ** (np.arange(0, HD, 2, dtype=np.float64) / HD))
    ang = np.arange(L, dtype=np.float64)[:, None] * inv[None, :]
    chalf = np.tile(np.cos(ang), (1, 4)).astype(F16)          # [L, 256]
    shalf = np.tile(np.sin(ang), (1, 4)).astype(F16)
    p = np.arange(128)[:, None]
    f = np.arange(128)[None, :]
    trimask = (p <= f).astype(F16)                             # [128, 128]

    in_maps = []
    for c in range(8):
        b, g = c % 4, c // 4
        qr = w_qkv[g * DL:(g + 1) * DL]
        kr = w_qkv[D + g * DL:D + (g + 1) * DL]
        vr = w_qkv[2 * D + g * DL:2 * D + (g + 1) * DL]
        wqkvT = np.ascontiguousarray(
            np.concatenate([qr[:512], kr[:512], vr[:512],
                            qr[512:], kr[512:], vr[512:]], axis=0).T
        ).astype(F16)
        in_maps.append({
            "xT": np.ascontiguousarray(x[b].T).astype(F16),
            "wqkvT": wqkvT,
            "woT": np.ascontiguousarray(
                w_o[:, g * DL:(g + 1) * DL].T).astype(F16),
            "chalf": chalf,
            "shalf": shalf,
            "trimask": trimask,
        })
    return in_maps


def kernel(x, w_qkv, w_o, _trace=False):
    x = np.asarray(x, dtype=np.float32)
    w_qkv = np.asarray(w_qkv, dtype=np.float32)
    w_o = np.asarray(w_o, dtype=np.float32)
    nc = _get_program()
    in_maps = _host_inputs(x, w_qkv, w_o)
    res = run_bass_kernel_spmd(nc, in_maps, core_ids=list(range(8)),
                               trace=_trace)
    kernel.last_result = res
    parts = [r["outT"] for r in res.results]
    out = np.empty((B, L, D), dtype=np.float32)
    for b in range(B):
        out[b] = (parts[b].astype(np.float32) +
                  parts[b + 4].astype(np.float32)).T
    return out



# revision 15
# speedup vs baseline: 1.1526x; 1.1526x over previous
"""MHA (RoPE + causal softmax attention + out-proj) on 8 NeuronCores.

Sharding: DP4 x TP2. Core c: batch b = c % 4, head-group g = c // 4
(8 heads per core). Each core computes a transposed partial output
outT = (y_local @ w_o_slice^T)^T in [D, L]; host sums the two head-group
partials per batch (fp16) and transposes back.

GEMM precision strategy: the two big dense GEMMs (QKV and out-proj) run
in fp8e4m3 DoubleRow mode (0.5 PE cycles/row while contracting 2x128
partitions = 4x the fp16 FLOP rate). Precision is recovered with an
error-compensated hi/lo split: a = a_hi + a_lo with both terms e4m3 at
the same power-of-2 scale, product computed as
  a@b ~= a_hi@b_hi + a_lo@b_hi + a_hi@b_lo   (x_lo@w_lo dropped)
which leaves ~1e-3 relative error (vs 2.7% for raw fp8) at 0.75x the
fp16 PE cost. All terms share one PSUM accumulation group because the
scales match; descales fold into existing evacuation ops (RoPE tables
untouched: q/k stay 2^12-scaled through DRAM and the score matmul, the
exp activation scale absorbs 2^-24). Attention itself stays fp16.
Weight splits are precomputed on host; the y split for the out-proj is
one extra Act copy + DVE sub per (head, q-chunk).

Layout strategy (unchanged from the fp16 version):
  Phase A: qkv natural layout [L, comps]; chunk 0 runs pair-outer over
           7 PSUM accumulators so matmuls chase the startup DMAs.
           RoPE applied with strided free-dim APs straight out of PSUM;
           rotated q/k staged to DRAM scratch (fp16) for the transposed
           reload; v copied (descaled) into SBUF group tiles.
  Phase B: per head, q/k loaded back transposed via DMA xbar transpose;
           scores computed transposed (k on partitions). exp on ScalarE
           with the combined 1/sqrt(HD)/2^24 scale fused. Causal handled
           exactly at 128-col granularity. Softmax denominator
           accumulated on DVE with a single all-ones(=1/32) matmul per
           (h, qc); finale deferred one block.
  Phase C: out-proj in fp8 DoubleRow; as soon as a q-chunk's 8 heads
           are split to fp8, its 16 out-proj tiles are queued and fed
           into later attention blocks as PE filler.
"""

import contextlib

import numpy as np
import ml_dtypes

import concourse.tile as tile
import concourse.mybir as mybir
from concourse import bacc
from concourse.bass_utils import run_bass_kernel_spmd

F16 = np.float16
NP8 = ml_dtypes.float8_e4m3
F32 = mybir.dt.float32
FP16 = mybir.dt.float16
FP8 = mybir.dt.float8e4
DR = mybir.MatmulPerfMode.DoubleRow

B, L, D, H, HD = 4, 2048, 2048, 16, 128
NH = 8                      # heads per core
DL = NH * HD                # 1024 local head dims
ROPE_BASE = 10000.0
ALPHA = float(HD) ** -0.5

S_X = 4.0                   # fp8 scale on x
S_W = 1024.0                # fp8 scale on w_qkv
SC = S_X * S_W              # 4096 = 2^12: scale carried by q/k/v psum
S_Y = 32.0                  # fp8 scale on y (attention output)
S_WO = 1024.0               # fp8 scale on w_o
EXPS = ALPHA / (SC * SC)    # fused exp scale (q and k each carry SC)
VDESC = 1.0 / SC            # v evacuation descale
ODESC = 1.0 / (S_Y * S_WO)  # out-proj evacuation descale

LT = L // 128               # 16 L-tiles
DT = D // 128               # 16 D(contract)-tiles
NPAIR = DT // 2             # 8 contraction pairs for DoubleRow
NCH = 6                     # qkv chunks of 512 comps: q03,k03,v03,q47,k47,v47
QC = L // 512               # 4 q-chunks of 512
KT = L // 128               # 16 k-tiles


def _chunk_kind(c):
    # chunk order: q(heads0-3), k(0-3), v(0-3), q(4-7), k(4-7), v(4-7)
    return ("q", "k", "v")[c % 3], c // 3


def build_program(la=5, scb=3):
    nc = bacc.Bacc("TRN2", target_bir_lowering=False, debug=False, num_devices=8)

    x8h = nc.dram_tensor("x8h", [NPAIR, 128, 2, L], FP8,
                         kind="ExternalInput").ap()
    x8l = nc.dram_tensor("x8l", [NPAIR, 128, 2, L], FP8,
                         kind="ExternalInput").ap()
    wq8h = nc.dram_tensor("wq8h", [NCH, 128, NPAIR, 2, 512], FP8,
                          kind="ExternalInput").ap()
    wq8l = nc.dram_tensor("wq8l", [NCH, 128, NPAIR, 2, 512], FP8,
                          kind="ExternalInput").ap()
    wo8h = nc.dram_tensor("wo8h", [4, 128, 2, D], FP8,
                          kind="ExternalInput").ap()
    wo8l = nc.dram_tensor("wo8l", [4, 128, 2, D], FP8,
                          kind="ExternalInput").ap()
    chalf = nc.dram_tensor("chalf", [L, 256], FP16, kind="ExternalInput").ap()
    shalf = nc.dram_tensor("shalf", [L, 256], FP16, kind="ExternalInput").ap()
    trimask = nc.dram_tensor("trimask", [128, 128], FP16, kind="ExternalInput").ap()
    outT = nc.dram_tensor("outT", [D, L], FP16, kind="ExternalOutput").ap()

    # DRAM staging for rotated q/k (natural layout, SC-scaled); v stays in SBUF
    qrot = nc.dram_tensor("qrot", [L, DL], FP16, kind="Internal").ap()
    krot = nc.dram_tensor("krot", [L, DL], FP16, kind="Internal").ap()

    with tile.TileContext(nc) as tc, contextlib.ExitStack() as es:
        pr = es.enter_context(tc.tile_pool(name="pR", bufs=1, side="right"))
        pqk03 = es.enter_context(
            tc.tile_pool(name="pQK03", bufs=1, side="right"))
        qkts03 = []
        qkts47 = []
        y8ts = [None] * 2       # [g] -> (hi, lo) tiles [128, 4, L] fp8
        vts = []
        wo8ts = []              # [(hi, lo)] * 4 d-pairs, [128, 2, D] fp8

        # ---------------- attention block emitter ----------------
        def attn_head_qc(pools, h, qc, pe_denom=False, dpool=None,
                         filler=None):
            """Emit attention for (head h, q-chunk qc); returns a finale
            closure (denominator reduce + normalize + fp8 split) the
            caller defers so PE never blocks on the DVE add-chain."""
            pss, pba, psy, psd, pbr = pools
            g, hl = h // 4, h % 4
            qt, kt = (qkts03 + qkts47)[h]
            nkt = 4 * qc + 4
            ypsum = psy.tile([128, 512], F32, name="ypsum", tag="yp")
            dacc = None
            if pe_denom:
                # accumulated on PE during the block; borrow a phase-C bank
                # (psD would WAR-deadlock against the deferred finales)
                dpsum = dpool()
            else:
                dpsum = psd.tile([128, 512], F32, name="dpsum", tag="dp")
                dacc = pbr.tile([128, 512], FP16, name="dacc", tag="dacc")
            ats = {}

            def emit_score(j):
                m = j - 4 * qc  # >= 0 on the diagonal block
                c0 = max(m, 0) * 128  # first valid within-chunk col
                sc = pss.tile([128, 512], F32, name="sc", tag="sc")
                nc.tensor.matmul(
                    sc[:, c0:], kt[:, j * 128:(j + 1) * 128],
                    qt[:, qc * 512 + c0:(qc + 1) * 512],
                    start=True, stop=True)
                at = pba.tile([128, 512], FP16, name="at", tag="at")
                nc.scalar.activation(
                    out=at[:, c0:], in_=sc[:, c0:],
                    func=mybir.ActivationFunctionType.Exp,
                    scale=EXPS)
                if m >= 0:
                    nc.vector.tensor_mul(
                        at[:, c0:c0 + 128], at[:, c0:c0 + 128], mt)
                ats[j] = at

            def emit_dadd_at(j, at):
                m = j - 4 * qc
                c0 = max(m, 0) * 128
                if pe_denom:
                    nc.tensor.matmul(
                        dpsum[:, c0:], ones128, at[:, c0:],
                        start=(j == 0), stop=(j == nkt - 1),
                        skip_group_check=True)
                elif j == 0:
                    nc.vector.tensor_copy(out=dacc, in_=at)
                else:
                    nc.vector.tensor_add(
                        dacc[:, c0:], dacc[:, c0:], at[:, c0:])

            n_off = 4 * qc
            next_emit = 0

            def emit_upto(n):
                nonlocal next_emit
                while next_emit < n:
                    emit_score(next_emit)
                    next_emit += 1

            emit_upto(min(la, nkt))
            if filler is not None:
                # PE work between the first scores and the first attn@V
                # consume hides the exp latency at block start
                filler()
            # off-diagonal k-tiles: full-width attn@V
            for j in range(n_off):
                emit_upto(min(j + 1 + la, nkt))
                at = ats.pop(j)
                nc.tensor.matmul(
                    ypsum, vts[g][:, j, hl * 128:(hl + 1) * 128], at,
                    start=(j == 0), stop=False)
                emit_dadd_at(j, at)
                if filler is not None and j % 2 == 1:
                    filler()

            # diagonal block: make sure all 4 at tiles exist first
            emit_upto(nkt)
            if filler is not None:
                filler()
                filler()
            d_ats = [ats.pop(4 * qc + m) for m in range(4)]
            for m in range(4):
                emit_dadd_at(4 * qc + m, d_ats[m])
            for mq in range(4):
                for m in range(mq + 1):
                    nc.tensor.matmul(
                        ypsum[:, mq * 128:(mq + 1) * 128],
                        vts[g][:, 4 * qc + m, hl * 128:(hl + 1) * 128],
                        d_ats[m][:, mq * 128:(mq + 1) * 128],
                        start=(qc == 0 and m == 0), stop=(m == mq),
                        skip_group_check=True)

            def finale():
                if not pe_denom:
                    nc.tensor.matmul(dpsum, ones128, dacc,
                                     start=True, stop=True)
                rbs = pbr.tile([128, 512], FP16, name="rbs", tag="rbs")
                with nc.allow_low_precision("softmax recip fp16"):
                    nc.vector.reciprocal(out=rbs, in_=dpsum)
                # y16 = S_Y * y (ones tile carries 1/S_Y), then hi/lo fp8.
                # dacc is dead once the denominator matmul has read it, so
                # reuse its slot as the y16 staging tile.
                y16 = dacc if dacc is not None else pbr.tile(
                    [128, 512], FP16, name="dacc", tag="dacc")
                nc.vector.tensor_mul(y16, ypsum, rbs)
                yh, yl = y8ts[g]
                nc.scalar.copy(out=yh[:, hl, qc * 512:(qc + 1) * 512],
                               in_=y16)
                nc.vector.tensor_sub(
                    yl[:, hl, qc * 512:(qc + 1) * 512], y16,
                    yh[:, hl, qc * 512:(qc + 1) * 512])

            return finale

        with tc.tile_pool(name="pAx", bufs=1) as pax, \
             tc.tile_pool(name="pAw", bufs=3) as paw:
            es_rope = contextlib.ExitStack()
            pcs = es_rope.enter_context(tc.tile_pool(name="pCs", bufs=1))
            pat = es_rope.enter_context(tc.tile_pool(name="pAt", bufs=2))
            pao = es_rope.enter_context(tc.tile_pool(name="pAo", bufs=6))
            # ------- resident left-side tiles + startup DMA order -------
            # DMA issue order is everything here: interleave per-pair hi
            # weight slices with the xh tiles term 1 needs, then the lo
            # halves, so the pair-outer matmuls chase the queue.
            wch0h = paw.tile([128, NPAIR, 2, 512], FP8, name="wchh",
                             tag="wchh")
            wch0l = paw.tile([128, NPAIR, 2, 512], FP8, name="wchl",
                             tag="wchl")
            xhs = []
            xls = []
            c_sb = pcs.tile([128, LT, 256], FP16, name="c_sb", tag="c_sb")
            s_sb = pcs.tile([128, LT, 256], FP16, name="s_sb", tag="s_sb")
            for j in range(NPAIR):
                nc.sync.dma_start(out=wch0h[:, j], in_=wq8h[0, :, j])
                xh = pax.tile([128, 2, L], FP8, name=f"xh{j}", tag=f"xh{j}")
                nc.sync.dma_start(out=xh, in_=x8h[j])
                xhs.append(xh)
            nc.sync.dma_start(out=wch0l, in_=wq8l[0])
            for j in range(NPAIR):
                xl = pax.tile([128, 2, L], FP8, name=f"xl{j}", tag=f"xl{j}")
                nc.sync.dma_start(out=xl, in_=x8l[j])
                xls.append(xl)
            # cos/sin after xl (first RoPE evac runs only once part 1 is
            # done) but before the chunk-1 weights (needed later still)
            nc.sync.dma_start(
                out=c_sb[:, 0:8, :],
                in_=chalf[0:1024, :].rearrange("(i p) g -> p i g", p=128))
            nc.sync.dma_start(
                out=s_sb[:, 0:8, :],
                in_=shalf[0:1024, :].rearrange("(i p) g -> p i g", p=128))
            nc.sync.dma_start(
                out=c_sb[:, 8:, :],
                in_=chalf[1024:, :].rearrange("(i p) g -> p i g", p=128))
            nc.sync.dma_start(
                out=s_sb[:, 8:, :],
                in_=shalf[1024:, :].rearrange("(i p) g -> p i g", p=128))

            mt = pr.tile([128, 128], FP16, name="trimask_sb", tag="mask")
            nc.sync.dma_start(out=mt, in_=trimask)
            ones128 = pr.tile([128, 128], FP16, name="ones128", tag="oc")
            nc.vector.memset(ones128, 1.0 / S_Y)

            for g in range(2):
                vt = pr.tile([128, LT, 512], FP16, name=f"vt{g}",
                             tag=f"vt{g}")
                vts.append(vt)

            def qkv_mms(pnat, i, wh, wl):
                """24 DoubleRow matmuls accumulating one [128,512] qkv
                tile: hi@hi, lo@hi, hi@lo over 8 contraction pairs."""
                s = slice(i * 128, (i + 1) * 128)
                for j in range(NPAIR):
                    nc.tensor.matmul(pnat, xhs[j][:, :, s], wh[:, j],
                                     start=(j == 0), stop=False,
                                     perf_mode=DR)
                for j in range(NPAIR):
                    nc.tensor.matmul(pnat, xls[j][:, :, s], wh[:, j],
                                     start=False, stop=False,
                                     perf_mode=DR)
                for j in range(NPAIR):
                    nc.tensor.matmul(pnat, xhs[j][:, :, s], wl[:, j],
                                     start=False, stop=(j == NPAIR - 1),
                                     perf_mode=DR)

            # ---------------- Phase A: QKV + RoPE ----------------
            def rope_evac(pnat, i, kind, grp):
                """Evacuate one [128,512] qkv psum tile.

                q/k path: deinterleave even/odd components on the Act
                engine (PSUM -> packed fp16 SBUF), then run the rotation
                on DVE entirely in packed fp16 so the 2x/4x DVE modes
                apply. Rotated components are stored PERMUTED per head
                ([64 evens | 64 odds] inside each head's 128-col block):
                the score contraction is invariant to component order as
                long as q and k share the permutation, and v / y are
                untouched."""
                if kind == "v":
                    nc.scalar.mul(out=vts[grp][:, i, :], in_=pnat,
                                  mul=VDESC)
                    return
                xx = pat.tile([128, 512], FP16, name="xx", tag="xx")
                nc.scalar.copy(out=xx[:, 0:256], in_=pnat[:, 0::2])
                nc.scalar.copy(out=xx[:, 256:512], in_=pnat[:, 1::2])
                x1 = xx[:, 0:256]
                x2 = xx[:, 256:512]
                ct = c_sb[:, i, :]
                st = s_sb[:, i, :]
                t1 = pat.tile([128, 256], FP16, name="t1", tag="t1")
                nc.vector.tensor_mul(t1, x1, ct)
                t2 = pat.tile([128, 256], FP16, name="t2", tag="t2")
                nc.vector.tensor_mul(t2, x2, st)
                t3 = pat.tile([128, 256], FP16, name="t3", tag="t3")
                nc.vector.tensor_mul(t3, x2, ct)
                t4 = pat.tile([128, 256], FP16, name="t4", tag="t4")
                nc.vector.tensor_mul(t4, x1, st)
                ro = pao.tile([128, 4, 2, 64], FP16, name="ro", tag="ro")
                def _h4(t):
                    return t.rearrange("p (h c) -> p h c", h=4)
                nc.vector.tensor_sub(ro[:, :, 0, :], _h4(t1), _h4(t2))
                nc.vector.tensor_add(ro[:, :, 1, :], _h4(t3), _h4(t4))
                dst = qrot if kind == "q" else krot
                # issue on the idle Pool engine's SWDGE queue: these writes
                # wait on the DVE chain and would head-of-line-block SP.SEQ
                # (delaying the chunk weight loads queued behind them)
                nc.gpsimd.dma_start(
                    out=dst[i * 128:(i + 1) * 128,
                            grp * 512:(grp + 1) * 512],
                    in_=ro.rearrange("p h t c -> p (h t c)"))

            # PSUM: psAV (2 banks, for the v47 chunk that overlaps phase B
            # pools) opens BEFORE psA6 so the pools can close in LIFO order
            # with no transition barrier at chunk 5.
            psav = es.enter_context(
                tc.tile_pool(name="psAV", bufs=1, space="PSUM"))

            def pv_tile(slot):
                return psav.tile([128, 512], F32, name="pv",
                                 tag=f"pv{slot}")

            avn = [0]

            def av_tile():
                t = pv_tile(avn[0] % 2)
                avn[0] += 1
                return t

            with tc.tile_pool(name="psA6", bufs=1, space="PSUM") as psa6:
                def pn_tile(slot):
                    return psa6.tile([128, 512], F32, name="pn",
                                     tag=f"pn{slot}")

                def c0_tile(i):
                    # chunk 0 borrows the psAV banks too: 8-deep rotation
                    s = i % 8
                    return pn_tile(s) if s < 6 else pv_tile(s - 6)

                # chunk 0 (q03) first part: pair-outer over 7 psum banks so
                # the matmuls chase the startup DMAs (term 1 needs only the
                # xh tiles; term 2 the xl tiles; term 3 the lo weights).
                pns = [c0_tile(ii) for ii in range(7)]
                for j in range(NPAIR):
                    for ii in range(7):
                        nc.tensor.matmul(
                            pns[ii],
                            xhs[j][:, :, ii * 128:(ii + 1) * 128],
                            wch0h[:, j], start=(j == 0), stop=False,
                            perf_mode=DR)
                for j in range(NPAIR):
                    for ii in range(7):
                        nc.tensor.matmul(
                            pns[ii],
                            xhs[j][:, :, ii * 128:(ii + 1) * 128],
                            wch0l[:, j], start=False, stop=False,
                            perf_mode=DR)
                for j in range(NPAIR):
                    for ii in range(7):
                        nc.tensor.matmul(
                            pns[ii],
                            xls[j][:, :, ii * 128:(ii + 1) * 128],
                            wch0h[:, j], start=False,
                            stop=(j == NPAIR - 1), perf_mode=DR)
                for ii in range(7):
                    rope_evac(pns[ii], ii, "q", 0)

                def load_wch(c, at_ms=None):
                    """Prefetch chunk c's weight pair one chunk ahead.
                    tile_wait_until parks the issue at roughly the right
                    sim time so the scheduler neither hoists it into the
                    startup queue (starving the critical x loads) nor
                    lets its WAR-wait seize SP.SEQ."""
                    with tc.tile_wait_until(at_ms, enable=at_ms is not None):
                        wh = paw.tile([128, NPAIR, 2, 512], FP8,
                                      name="wchh", tag="wchh")
                        nc.sync.dma_start(out=wh, in_=wq8h[c])
                        wl = paw.tile([128, NPAIR, 2, 512], FP8,
                                      name="wchl", tag="wchl")
                        nc.sync.dma_start(out=wl, in_=wq8l[c])
                    return wh, wl

                wnext = load_wch(1)
                # chunk 0 rest: data resident, i-outer so the RoPE
                # evacuations drain while the matmuls continue.
                for i in range(7, LT):
                    pnat = c0_tile(i)
                    qkv_mms(pnat, i, wch0h, wch0l)
                    rope_evac(pnat, i, "q", 0)

                # chunks 1-4: i-outer, pair-inner (weights triple-buffered),
                # cycling through the 6 psA6 banks.
                for c in range(1, NCH - 1):
                    kind, grp = _chunk_kind(c)
                    wh, wl = wnext
                    wnext = load_wch(c + 1, at_ms=0.053 + (c - 1) * 0.047)
                    for i in range(LT):
                        pnat = pn_tile(i % 6)
                        qkv_mms(pnat, i, wh, wl)
                        rope_evac(pnat, i, kind, grp)
                    if c == 1:
                        # rotated q/k for heads 0-3 are now staged in
                        # DRAM: issue their transposed reloads early so
                        # they drain during the rest of phase A.
                        for h in range(4):
                            qt = pqk03.tile([128, L], FP16, name="qt03",
                                            tag=f"qt{h}", bufs=1)
                            nc.sync.dma_start_transpose(
                                out=qt,
                                in_=qrot[:, h * 128:(h + 1) * 128])
                            kt = pqk03.tile([128, L], FP16, name="kt03",
                                            tag=f"kt{h}", bufs=1)
                            nc.sync.dma_start_transpose(
                                out=kt,
                                in_=krot[:, h * 128:(h + 1) * 128])
                            qkts03.append((qt, kt))

            # RoPE pools (cos/sin, temps, psA8) close here; open the
            # attention pools that must outlive phase A, then emit chunk 5
            # (v47, no RoPE) with (heads 0-3, qc 0) attention interleaved.
            es_rope.close()
            pba = es.enter_context(
                tc.tile_pool(name="pBa", bufs=10, side="right"))
            pbr = es.enter_context(
                tc.tile_pool(name="pBr", bufs=2, side="right"))
            pyts = es.enter_context(
                tc.tile_pool(name="pYts", bufs=1, side="right"))
            pss = es.enter_context(
                tc.tile_pool(name="psS", bufs=scb, space="PSUM"))
            psy = es.enter_context(
                tc.tile_pool(name="psY", bufs=2, space="PSUM"))
            psd = es.enter_context(
                tc.tile_pool(name="psD", bufs=1, space="PSUM"))
            y8ts[0] = (pyts.tile([128, 4, L], FP8, name="y03h", tag="y03h"),
                       pyts.tile([128, 4, L], FP8, name="y03l", tag="y03l"))
            bpools = (pss, pba, psy, psd, pbr)
            pending = None
            c = NCH - 1
            kind, grp = _chunk_kind(c)
            wh, wl = wnext
            inject = {4: (0, 0), 8: (0, 1), 12: (0, 2), 15: (0, 3)}
            for i in range(LT):
                pnat = pv_tile(i % 2)
                qkv_mms(pnat, i, wh, wl)
                rope_evac(pnat, i, kind, grp)
                if i in inject:
                    qci, hi = inject[i]
                    fin = attn_head_qc(bpools, hi, qci, dpool=av_tile)
                    if pending is not None:
                        pending()
                    pending = fin

        # ---------------- Phase B rest + Phase C ----------------
        pqkb = es.enter_context(
            tc.tile_pool(name="pQK47", bufs=1, side="right"))
        pcw = es.enter_context(tc.tile_pool(name="pCw", bufs=1))
        pco = es.enter_context(tc.tile_pool(name="pCo", bufs=4))
        for h in range(4, NH):
            qt = pqkb.tile([128, L], FP16, name="qt47", tag=f"qt{h}")
            nc.sync.dma_start_transpose(
                out=qt, in_=qrot[:, h * 128:(h + 1) * 128])
            kt = pqkb.tile([128, L], FP16, name="kt47", tag=f"kt{h}")
            nc.sync.dma_start_transpose(
                out=kt, in_=krot[:, h * 128:(h + 1) * 128])
            qkts47.append((qt, kt))
        y8ts[1] = (pqkb.tile([128, 4, L], FP8, name="y47h", tag="y47h"),
                   pqkb.tile([128, 4, L], FP8, name="y47l", tag="y47l"))
        for m in range(4):
            woh = pcw.tile([128, 2, D], FP8, name=f"woh{m}", tag=f"woh{m}")
            nc.sync.dma_start(out=woh, in_=wo8h[m])
            wol = pcw.tile([128, 2, D], FP8, name=f"wol{m}", tag=f"wol{m}")
            nc.sync.dma_start(out=wol, in_=wo8l[m])
            wo8ts.append((woh, wol))

        # remaining blocks: interleave qc1 heads 0-3 (their q/k are
        # resident) with qc0 heads 4-7 so each block waits at most one
        # in-flight transposed reload.
        seq = [(1, 0), (1, 1), (0, 4), (0, 5), (0, 6), (0, 7),
               (1, 2), (1, 3), (1, 4), (1, 5), (1, 6), (1, 7)]
        seq += [(qc, h) for qc in (2, 3) for h in range(NH)]

        # phase C borrows the psAV banks (freed after chunk 5); pools on
        # the PSUM side must close LIFO so psAV simply stays open.
        def _ymov(m, hilo, qc):
            """Moving operand for out-proj pair m: heads (2m, 2m+1)."""
            t = y8ts[m // 2][hilo]
            r = (2 * m) % 4
            return t[:, r:r + 2, qc * 512:(qc + 1) * 512]

        def c_tile(qc, e):
            op = av_tile()
            se = slice(e * 128, (e + 1) * 128)
            for m in range(4):
                nc.tensor.matmul(op, wo8ts[m][0][:, :, se], _ymov(m, 0, qc),
                                 start=(m == 0), stop=False, perf_mode=DR)
            for m in range(4):
                nc.tensor.matmul(op, wo8ts[m][0][:, :, se], _ymov(m, 1, qc),
                                 start=False, stop=False, perf_mode=DR)
            for m in range(4):
                nc.tensor.matmul(op, wo8ts[m][1][:, :, se], _ymov(m, 0, qc),
                                 start=False, stop=(m == 3), perf_mode=DR)
            ot = pco.tile([128, 512], FP16, name="ot", tag="ot")
            nc.scalar.mul(out=ot, in_=op, mul=ODESC)
            nc.sync.dma_start(
                out=outT[e * 128:(e + 1) * 128,
                         qc * 512:(qc + 1) * 512],
                in_=ot)

        cwork = []
        quota = [0]
        since_refill = [0]

        def filler():
            if cwork and quota[0] > 0:
                quota[0] -= 1
                cwork.pop(0)()

        emitted_fin = {0: 3, 1: 0, 2: 0, 3: 0}  # h0-h2 flushed in A
        for k, (qc, h) in enumerate(seq):
            since_refill[0] += 1
            bl = max(1, 8 - since_refill[0])
            quota[0] = (len(cwork) + bl - 1) // bl if cwork else 0
            last = k == len(seq) - 1
            fin = attn_head_qc(bpools, h, qc, pe_denom=last,
                               dpool=av_tile, filler=filler)
            if pending is not None:
                pending()
                fq = seq[k - 1] if k > 0 else (0, 3)
                emitted_fin[fq[0]] += 1
                if emitted_fin[fq[0]] == NH and fq[0] < QC - 1:
                    cwork.extend(
                        (lambda qq, ee: lambda: c_tile(qq, ee))(
                            fq[0], e) for e in range(DT))
                    since_refill[0] = 0
            if last:
                # all-PE denominator: the finale is cheap, flush it
                # now so phase C can be emitted last
                fin()
                pending = None
            else:
                pending = fin
        quota[0] = len(cwork)
        while cwork:
            filler()
        for e in range(DT):
            c_tile(QC - 1, e)
    nc.compile()
    return nc


_NC_CACHE = None


def _get_program():
    global _NC_CACHE
    if _NC_CACHE is None:
        _NC_CACHE = build_program()
    return _NC_CACHE


def _split8(a):
    """Hi/lo e4m3 split of an (already scaled) fp32 array."""
    h = a.astype(NP8)
    l = (a - h.astype(np.float32)).astype(NP8)
    return h, l


def _host_inputs(x, w_qkv, w_o):
    inv = 1.0 / (ROPE_BASE ** (np.arange(0, HD, 2, dtype=np.float64) / HD))
    ang = np.arange(L, dtype=np.float64)[:, None] * inv[None, :]
    chalf = np.tile(np.cos(ang), (1, 4)).astype(F16)          # [L, 256]
    shalf = np.tile(np.sin(ang), (1, 4)).astype(F16)
    p = np.arange(128)[:, None]
    f = np.arange(128)[None, :]
    trimask = (p <= f).astype(F16)                             # [128, 128]

    # per-batch x splits: [NPAIR, 128, 2, L] fp8
    xs = []
    for b in range(B):
        xT = np.ascontiguousarray(x[b].T) * np.float32(S_X)
        h8, l8 = _split8(xT)
        xs.append((
            np.ascontiguousarray(
                h8.reshape(NPAIR, 2, 128, L).transpose(0, 2, 1, 3)),
            np.ascontiguousarray(
                l8.reshape(NPAIR, 2, 128, L).transpose(0, 2, 1, 3)),
        ))

    # per-group w_qkv splits: [NCH, 128, NPAIR, 2, 512] fp8
    ws = []
    wos = []
    for g in range(2):
        qr = w_qkv[g * DL:(g + 1) * DL]
        kr = w_qkv[D + g * DL:D + (g + 1) * DL]
        vr = w_qkv[2 * D + g * DL:2 * D + (g + 1) * DL]
        wqkvT = np.ascontiguousarray(
            np.concatenate([qr[:512], kr[:512], vr[:512],
                            qr[512:], kr[512:], vr[512:]], axis=0).T
        ) * np.float32(S_W)                                    # [D, 3DL]
        h8, l8 = _split8(wqkvT)

        def _wlay(a):
            return np.ascontiguousarray(
                a.reshape(NPAIR, 2, 128, NCH, 512).transpose(3, 2, 0, 1, 4))
        ws.append((_wlay(h8), _wlay(l8)))

        woT = np.ascontiguousarray(
            w_o[:, g * DL:(g + 1) * DL].T) * np.float32(S_WO)  # [DL, D]
        h8, l8 = _split8(woT)

        def _olay(a):
            return np.ascontiguousarray(
                a.reshape(4, 2, 128, D).transpose(0, 2, 1, 3))
        wos.append((_olay(h8), _olay(l8)))

    in_maps = []
    for c in range(8):
        b, g = c % 4, c // 4
        in_maps.append({
            "x8h": xs[b][0],
            "x8l": xs[b][1],
            "wq8h": ws[g][0],
            "wq8l": ws[g][1],
            "wo8h": wos[g][0],
            "wo8l": wos[g][1],
            "chalf": chalf,
            "shalf": shalf,
            "trimask": trimask,
        })
    return in_maps


def kernel(x, w_qkv, w_o, _trace=False):
    x = np.asarray(x, dtype=np.float32)
    w_qkv = np.asarray(w_qkv, dtype=np.float32)
    w_o = np.asarray(w_o, dtype=np.float32)
    nc = _get_program()
    in_maps = _host_inputs(x, w_qkv, w_o)
    res = run_bass_kernel_spmd(nc, in_maps, core_ids=list(range(8)),
                               trace=_trace)
    kernel.last_result = res
    parts = [r["outT"] for r in res.results]
    out = np.empty((B, L, D), dtype=np.float32)
    for b in range(B):
        out[b] = (parts[b].astype(np.float32) +
                  parts[b + 4].astype(np.float32)).T
    return out


# revision 43
# speedup vs baseline: 1.2259x; 1.0636x over previous
"""MHA (RoPE + causal softmax attention + out-proj) on 8 NeuronCores.

Sharding: DP4 x TP2. Core c: batch b = c % 4, head-group g = c // 4
(8 heads per core). Each core computes a transposed partial output
outT = (y_local @ w_o_slice^T)^T in [D, L]; host sums the two head-group
partials per batch (fp16) and transposes back.

GEMM precision strategy: the two big dense GEMMs (QKV and out-proj) run
in fp8e4m3 DoubleRow mode (0.5 PE cycles/row while contracting 2x128
partitions = 4x the fp16 FLOP rate). Precision is recovered with an
error-compensated hi/lo split: a = a_hi + a_lo with both terms e4m3 at
the same power-of-2 scale, product computed as
  a@b ~= a_hi@b_hi + a_lo@b_hi + a_hi@b_lo   (x_lo@w_lo dropped)
which leaves ~1e-3 relative error (vs 2.7% for raw fp8) at 0.75x the
fp16 PE cost. All terms share one PSUM accumulation group because the
scales match; descales fold into existing evacuation ops (RoPE tables
untouched: q/k stay 2^12-scaled through DRAM and the score matmul, the
exp activation scale absorbs 2^-24). Attention itself stays fp16.
Weight splits are precomputed on host; the y split for the out-proj is
one extra Act copy + DVE sub per (head, q-chunk).

Layout / schedule strategy:
  Phase A: qkv natural layout [L, comps], chunks ordered v03, q03, k03,
           q47, k47, v47 (v first keeps cos/sin and the rope-write DMAs
           off the startup critical path; v last gives the attention
           injections an Act-light chunk to hide under). Chunk 0 runs
           pair-outer over 7 PSUM banks chasing the startup DMA queue,
           computing only the x_hi terms (the wl term lagged one pair
           behind the wh term to match arrival order): the x_lo@w_hi
           contribution is DEFERRED into chunk 1 (psAV banks idle
           there) and accumulated into the live v tiles with one fused
           DVE (psum*scale + v) op per tile, which removes the 4MB of
           x_lo from the startup-critical DMA supply entirely.
           RoPE evac: even/odd components deinterleaved by two Act
           copies (PSUM -> packed fp16 SBUF), rotation on DVE in packed
           fp16 (2x modes), components stored PERMUTED per head
           ([64 evens | 64 odds] per 128-col block — the score
           contraction is invariant since q and k share the
           permutation). Rotated q/k staged to DRAM (fp16, 2^12-scaled)
           for the transposed reload; rope writes and chunk>=2 weight
           prefetches issue on the Pool SWDGE queue so their waits
           cannot head-of-line-block SP.SEQ.
  Phase B: per head, q/k reloaded transposed via DMA xbar transpose;
           scores computed transposed (k on partitions). exp on ScalarE
           with the 1/sqrt(HD)/2^24 scale fused. Causal handled exactly
           at 128-col granularity. Softmax denominator split into two
           partial accumulators (even k-tiles on DVE, odd on GPSIMD) so
           neither add-chain gates the block; the partials merge with
           one DVE add so a single all-ones(=1/32) matmul per (h, qc)
           reduces partitions. All 8 qc0/qc1 blocks
           of heads 0-3 are injected under the v47 chunk; the remaining
           sequence leads with resident-q/k qc2 blocks to cover the
           qk47 transpose latency. Finales deferred one block.
  Phase C: out-proj in fp8 DoubleRow; as soon as a q-chunk's 8 heads
           are split to fp8, its 16 out-proj tiles are queued and fed
           into later attention blocks as PE filler (DVE evacuation —
           the tail stretch is Act-bound). The last q-chunk's tiles
           rotate over 5 PSUM banks freed by attention.
"""

import contextlib

import numpy as np
import ml_dtypes

import concourse.tile as tile
import concourse.mybir as mybir
from concourse import bacc
from concourse.bass_utils import run_bass_kernel_spmd

F16 = np.float16
NP8 = ml_dtypes.float8_e4m3
F32 = mybir.dt.float32
FP16 = mybir.dt.float16
FP8 = mybir.dt.float8e4
DR = mybir.MatmulPerfMode.DoubleRow

B, L, D, H, HD = 4, 2048, 2048, 16, 128
NH = 8                      # heads per core
DL = NH * HD                # 1024 local head dims
ROPE_BASE = 10000.0
ALPHA = float(HD) ** -0.5

S_X = 4.0                   # fp8 scale on x
S_W = 1024.0                # fp8 scale on w_qkv
SC = S_X * S_W              # 4096 = 2^12: scale carried by q/k/v psum
S_Y = 32.0                  # fp8 scale on y (attention output)
S_WO = 1024.0               # fp8 scale on w_o
EXPS = ALPHA / (SC * SC)    # fused exp scale (q and k each carry SC)
VDESC = 1.0 / SC            # v evacuation descale
ODESC = 1.0 / (S_Y * S_WO)  # out-proj evacuation descale

LT = L // 128               # 16 L-tiles
DT = D // 128               # 16 D(contract)-tiles
NPAIR = DT // 2             # 8 contraction pairs for DoubleRow
NCH = 6                     # qkv chunks of 512 comps: q03,k03,v03,q47,k47,v47
QC = L // 512               # 4 q-chunks of 512
KT = L // 128               # 16 k-tiles


def _chunk_kind(c):
    # chunk order: v03, q03, k03, q47, k47, v47 — v first so the startup
    # path needs neither cos/sin nor rope writes; v last so the attention
    # injections overlap an Act-only evac chunk
    return [("v", 0), ("q", 0), ("k", 0),
            ("q", 1), ("k", 1), ("v", 1)][c]


def build_program(la=5, scb=3):
    nc = bacc.Bacc("TRN2", target_bir_lowering=False, debug=False, num_devices=8)

    x8h = nc.dram_tensor("x8h", [NPAIR, 128, 2, L], FP8,
                         kind="ExternalInput").ap()
    x8l = nc.dram_tensor("x8l", [NPAIR, 128, 2, L], FP8,
                         kind="ExternalInput").ap()
    wq8h = nc.dram_tensor("wq8h", [NCH, 128, NPAIR, 2, 512], FP8,
                          kind="ExternalInput").ap()
    wq8l = nc.dram_tensor("wq8l", [NCH, 128, NPAIR, 2, 512], FP8,
                          kind="ExternalInput").ap()
    wo8h = nc.dram_tensor("wo8h", [4, 128, 2, D], FP8,
                          kind="ExternalInput").ap()
    wo8l = nc.dram_tensor("wo8l", [4, 128, 2, D], FP8,
                          kind="ExternalInput").ap()
    chalf = nc.dram_tensor("chalf", [L, 256], FP16, kind="ExternalInput").ap()
    shalf = nc.dram_tensor("shalf", [L, 256], FP16, kind="ExternalInput").ap()
    trimask = nc.dram_tensor("trimask", [128, 128], FP16, kind="ExternalInput").ap()
    outT = nc.dram_tensor("outT", [D, L], FP16, kind="ExternalOutput").ap()

    # DRAM staging for rotated q/k (natural layout, SC-scaled); v stays in SBUF
    qrot = nc.dram_tensor("qrot", [L, DL], FP16, kind="Internal").ap()
    krot = nc.dram_tensor("krot", [L, DL], FP16, kind="Internal").ap()

    with tile.TileContext(nc) as tc, contextlib.ExitStack() as es:
        pr = es.enter_context(tc.tile_pool(name="pR", bufs=1, side="right"))
        pqk03 = es.enter_context(
            tc.tile_pool(name="pQK03", bufs=1, side="right"))
        qkts03 = []
        qkts47 = []
        y8ts = [None] * 2       # [g] -> (hi, lo) tiles [128, 4, L] fp8
        vts = []
        wo8ts = []              # [(hi, lo)] * 4 d-pairs, [128, 2, D] fp8

        # ---------------- attention block emitter ----------------
        def attn_head_qc(pools, h, qc, pe_denom=False, dpool=None,
                         filler=None):
            """Emit attention for (head h, q-chunk qc); returns a finale
            closure (denominator reduce + normalize + fp8 split) the
            caller defers so PE never blocks on the DVE add-chain."""
            pss, pba, psy, psd, pbr = pools
            g, hl = h // 4, h % 4
            qt, kt = (qkts03 + qkts47)[h]
            nkt = 4 * qc + 4
            ypsum = psy.tile([128, 512], F32, name="ypsum", tag="yp")
            dacc = None
            if pe_denom:
                # accumulated on PE during the block; borrow a phase-C bank
                # (psD would WAR-deadlock against the deferred finales)
                dpsum = dpool()
            else:
                dpsum = psd.tile([128, 512], F32, name="dpsum", tag="dp")
                dacc = pbr.tile([128, 512], FP16, name="dacc", tag="dacc")
                # second partial accumulator on the (idle) Pool engine:
                # halves the serial DVE add-chain per block
                dacc2 = pbr.tile([128, 512], FP16, name="dacc2",
                                 tag="dacc2")
            ats = {}

            def emit_score(j):
                m = j - 4 * qc  # >= 0 on the diagonal block
                c0 = max(m, 0) * 128  # first valid within-chunk col
                sc = pss.tile([128, 512], F32, name="sc", tag="sc")
                nc.tensor.matmul(
                    sc[:, c0:], kt[:, j * 128:(j + 1) * 128],
                    qt[:, qc * 512 + c0:(qc + 1) * 512],
                    start=True, stop=True)
                at = pba.tile([128, 512], FP16, name="at", tag="at")
                nc.scalar.activation(
                    out=at[:, c0:], in_=sc[:, c0:],
                    func=mybir.ActivationFunctionType.Exp,
                    scale=EXPS)
                if m >= 0:
                    nc.vector.tensor_mul(
                        at[:, c0:c0 + 128], at[:, c0:c0 + 128], mt)
                ats[j] = at

            def emit_dadd_at(j, at):
                m = j - 4 * qc
                c0 = max(m, 0) * 128
                if pe_denom:
                    nc.tensor.matmul(
                        dpsum[:, c0:], ones128, at[:, c0:],
                        start=(j == 0), stop=(j == nkt - 1),
                        skip_group_check=True)
                elif j == 0:
                    nc.vector.tensor_copy(out=dacc, in_=at)
                elif j == 1:
                    # for qc==0 this is a diagonal tile: only [c0:] is
                    # initialized; the finale's dacc2 matmul is trimmed
                    # to match
                    nc.gpsimd.tensor_copy(out=dacc2[:, c0:],
                                          in_=at[:, c0:])
                elif j % 2 == 0:
                    nc.vector.tensor_add(
                        dacc[:, c0:], dacc[:, c0:], at[:, c0:])
                else:
                    nc.gpsimd.tensor_add(
                        dacc2[:, c0:], dacc2[:, c0:], at[:, c0:])

            n_off = 4 * qc
            next_emit = 0

            def emit_upto(n):
                nonlocal next_emit
                while next_emit < n:
                    emit_score(next_emit)
                    next_emit += 1

            emit_upto(min(la, nkt))
            if filler is not None:
                # PE work between the first scores and the first attn@V
                # consume hides the exp latency at block start
                filler()
            # off-diagonal k-tiles: full-width attn@V
            for j in range(n_off):
                emit_upto(min(j + 1 + la, nkt))
                at = ats.pop(j)
                nc.tensor.matmul(
                    ypsum, vts[g][:, j, hl * 128:(hl + 1) * 128], at,
                    start=(j == 0), stop=False)
                emit_dadd_at(j, at)
                if filler is not None and j % 2 == 1:
                    filler()

            # diagonal block: make sure all 4 at tiles exist first
            emit_upto(nkt)
            if filler is not None:
                filler()
                filler()
            d_ats = [ats.pop(4 * qc + m) for m in range(4)]
            for m in range(4):
                emit_dadd_at(4 * qc + m, d_ats[m])
            for mq in range(4):
                for m in range(mq + 1):
                    nc.tensor.matmul(
                        ypsum[:, mq * 128:(mq + 1) * 128],
                        vts[g][:, 4 * qc + m, hl * 128:(hl + 1) * 128],
                        d_ats[m][:, mq * 128:(mq + 1) * 128],
                        start=(qc == 0 and m == 0), stop=(m == mq),
                        skip_group_check=True)

            def finale():
                if not pe_denom:
                    # fold the Pool-side partial into dacc on DVE so the
                    # partition reduction needs a single PE matmul
                    co2 = 128 if qc == 0 else 0
                    nc.vector.tensor_add(dacc[:, co2:], dacc[:, co2:],
                                         dacc2[:, co2:])
                    nc.tensor.matmul(dpsum, ones128, dacc,
                                     start=True, stop=True)
                rbs = pbr.tile([128, 512], FP16, name="rbs", tag="rbs")
                with nc.allow_low_precision("softmax recip fp16"):
                    nc.vector.reciprocal(out=rbs, in_=dpsum)
                # y16 = S_Y * y (ones tile carries 1/S_Y), then hi/lo fp8.
                # dacc is dead once the denominator matmul has read it, so
                # reuse its slot as the y16 staging tile.
                y16 = dacc if dacc is not None else pbr.tile(
                    [128, 512], FP16, name="dacc", tag="dacc")
                nc.vector.tensor_mul(y16, ypsum, rbs)
                yh, yl = y8ts[g]
                nc.scalar.copy(out=yh[:, hl, qc * 512:(qc + 1) * 512],
                               in_=y16)
                nc.vector.tensor_sub(
                    yl[:, hl, qc * 512:(qc + 1) * 512], y16,
                    yh[:, hl, qc * 512:(qc + 1) * 512])

            return finale

        with tc.tile_pool(name="pAx", bufs=1) as pax, \
             tc.tile_pool(name="pAw", bufs=3) as paw:
            es_rope = contextlib.ExitStack()
            pcs = es_rope.enter_context(tc.tile_pool(name="pCs", bufs=1))
            pat = es_rope.enter_context(tc.tile_pool(name="pAt", bufs=2))
            pao = es_rope.enter_context(tc.tile_pool(name="pAo", bufs=6))
            # ------- resident left-side tiles + startup DMA order -------
            # DMA issue order is everything here: interleave per-pair hi
            # weight slices with the xh tiles term 1 needs, then the lo
            # halves, so the pair-outer matmuls chase the queue.
            wch0h = paw.tile([128, NPAIR, 2, 512], FP8, name="wchh",
                             tag="wchh")
            wch0l = paw.tile([128, NPAIR, 2, 512], FP8, name="wchl",
                             tag="wchl")
            xhs = []
            xls = []
            c_sb = pcs.tile([128, LT, 256], FP16, name="c_sb", tag="c_sb")
            s_sb = pcs.tile([128, LT, 256], FP16, name="s_sb", tag="s_sb")
            for j in range(NPAIR):
                nc.sync.dma_start(out=wch0h[:, j], in_=wq8h[0, :, j])
                xh = pax.tile([128, 2, L], FP8, name=f"xh{j}", tag=f"xh{j}")
                nc.sync.dma_start(out=xh, in_=x8h[j])
                xhs.append(xh)
                if j == 0:
                    nc.sync.dma_start(out=wch0l, in_=wq8l[0])
            with tc.tile_wait_until(0.030):
                for j in range(NPAIR):
                    xl = pax.tile([128, 2, L], FP8, name=f"xl{j}",
                                  tag=f"xl{j}")
                    nc.sync.dma_start(out=xl, in_=x8l[j])
                    xls.append(xl)
            # cos/sin after xl (first RoPE evac runs only once part 1 is
            # done) but before the chunk-1 weights (needed later still)
            nc.sync.dma_start(
                out=c_sb[:, 0:8, :],
                in_=chalf[0:1024, :].rearrange("(i p) g -> p i g", p=128))
            nc.sync.dma_start(
                out=s_sb[:, 0:8, :],
                in_=shalf[0:1024, :].rearrange("(i p) g -> p i g", p=128))
            nc.sync.dma_start(
                out=c_sb[:, 8:, :],
                in_=chalf[1024:, :].rearrange("(i p) g -> p i g", p=128))
            nc.sync.dma_start(
                out=s_sb[:, 8:, :],
                in_=shalf[1024:, :].rearrange("(i p) g -> p i g", p=128))

            mt = pr.tile([128, 128], FP16, name="trimask_sb", tag="mask")
            nc.sync.dma_start(out=mt, in_=trimask)
            ones128 = pr.tile([128, 128], FP16, name="ones128", tag="oc")
            nc.vector.memset(ones128, 1.0 / S_Y)

            for g in range(2):
                vt = pr.tile([128, LT, 512], FP16, name=f"vt{g}",
                             tag=f"vt{g}")
                vts.append(vt)

            def qkv_mms_hi(pnat, i):
                """Chunk-0 (v03) tile with only the x_hi terms: the
                x_lo@w_hi correction is deferred into chunk 1 so the xl
                tiles leave the startup critical path entirely."""
                s = slice(i * 128, (i + 1) * 128)
                for j in range(NPAIR):
                    nc.tensor.matmul(pnat, xhs[j][:, :, s], wch0h[:, j],
                                     start=(j == 0), stop=False,
                                     perf_mode=DR)
                for j in range(NPAIR):
                    nc.tensor.matmul(pnat, xhs[j][:, :, s], wch0l[:, j],
                                     start=False, stop=(j == NPAIR - 1),
                                     perf_mode=DR)

            def v03_fix(i, slot):
                """Deferred x_lo@w_hi contribution for v03 tile i,
                accumulated into the live vts[0] slice (vts += psum/SC).
                Runs on the psAV banks, which idle during chunks 1-4."""
                vc = pv_tile(slot % 2)
                sfix = slice(i * 128, (i + 1) * 128)
                for j in range(NPAIR):
                    nc.tensor.matmul(vc, xls[j][:, :, sfix], wch0h[:, j],
                                     start=(j == 0),
                                     stop=(j == NPAIR - 1), perf_mode=DR)
                nc.vector.scalar_tensor_tensor(
                    out=vts[0][:, i, :], in0=vc, scalar=VDESC,
                    in1=vts[0][:, i, :], op0=mybir.AluOpType.mult,
                    op1=mybir.AluOpType.add)

            def qkv_mms(pnat, i, wh, wl):
                """24 DoubleRow matmuls accumulating one [128,512] qkv
                tile: hi@hi, lo@hi, hi@lo over 8 contraction pairs."""
                s = slice(i * 128, (i + 1) * 128)
                for j in range(NPAIR):
                    nc.tensor.matmul(pnat, xhs[j][:, :, s], wh[:, j],
                                     start=(j == 0), stop=False,
                                     perf_mode=DR)
                for j in range(NPAIR):
                    nc.tensor.matmul(pnat, xls[j][:, :, s], wh[:, j],
                                     start=False, stop=False,
                                     perf_mode=DR)
                for j in range(NPAIR):
                    nc.tensor.matmul(pnat, xhs[j][:, :, s], wl[:, j],
                                     start=False, stop=(j == NPAIR - 1),
                                     perf_mode=DR)

            # ---------------- Phase A: QKV + RoPE ----------------
            def rope_evac(pnat, i, kind, grp):
                """Evacuate one [128,512] qkv psum tile.

                q/k path: deinterleave even/odd components on the Act
                engine (PSUM -> packed fp16 SBUF), then run the rotation
                on DVE entirely in packed fp16 so the 2x/4x DVE modes
                apply. Rotated components are stored PERMUTED per head
                ([64 evens | 64 odds] inside each head's 128-col block):
                the score contraction is invariant to component order as
                long as q and k share the permutation, and v / y are
                untouched."""
                if kind == "v":
                    nc.scalar.mul(out=vts[grp][:, i, :], in_=pnat,
                                  mul=VDESC)
                    return
                xx = pat.tile([128, 512], FP16, name="xx", tag="xx")
                # single strided-AP copy deinterleaves both halves at once
                # (one Act init penalty instead of two)
                nc.scalar.copy(
                    out=xx.rearrange("p (t c) -> p t c", t=2),
                    in_=pnat.rearrange("p (c t) -> p t c", t=2))
                x1 = xx[:, 0:256]
                x2 = xx[:, 256:512]
                ct = c_sb[:, i, :]
                st = s_sb[:, i, :]
                t1 = pat.tile([128, 256], FP16, name="t1", tag="t1")
                nc.vector.tensor_mul(t1, x1, ct)
                t2 = pat.tile([128, 256], FP16, name="t2", tag="t2")
                nc.vector.tensor_mul(t2, x2, st)
                t3 = pat.tile([128, 256], FP16, name="t3", tag="t3")
                nc.vector.tensor_mul(t3, x2, ct)
                t4 = pat.tile([128, 256], FP16, name="t4", tag="t4")
                nc.vector.tensor_mul(t4, x1, st)
                ro = pao.tile([128, 4, 2, 64], FP16, name="ro", tag="ro")
                def _h4(t):
                    return t.rearrange("p (h c) -> p h c", h=4)
                nc.vector.tensor_sub(ro[:, :, 0, :], _h4(t1), _h4(t2))
                nc.vector.tensor_add(ro[:, :, 1, :], _h4(t3), _h4(t4))
                dst = qrot if kind == "q" else krot
                # issue on the idle Pool engine's SWDGE queue: these writes
                # wait on the DVE chain and would head-of-line-block SP.SEQ
                # (delaying the chunk weight loads queued behind them)
                nc.gpsimd.dma_start(
                    out=dst[i * 128:(i + 1) * 128,
                            grp * 512:(grp + 1) * 512],
                    in_=ro.rearrange("p h t c -> p (h t c)"))

            # PSUM: psAV (2 banks, for the v47 chunk that overlaps phase B
            # pools) opens BEFORE psA6 so the pools can close in LIFO order
            # with no transition barrier at chunk 5.
            psav = es.enter_context(
                tc.tile_pool(name="psAV", bufs=1, space="PSUM"))

            def pv_tile(slot):
                return psav.tile([128, 512], F32, name="pv",
                                 tag=f"pv{slot}")

            avn = [0]

            def av_tile():
                t = pv_tile(avn[0] % 2)
                avn[0] += 1
                return t

            with tc.tile_pool(name="psA6", bufs=1, space="PSUM") as psa6:
                def pn_tile(slot):
                    return psa6.tile([128, 512], F32, name="pn",
                                     tag=f"pn{slot}")

                def c0_tile(i):
                    # chunk 0 borrows the psAV banks too: 8-deep rotation
                    s = i % 8
                    return pn_tile(s) if s < 6 else pv_tile(s - 6)

                # chunk 0 (q03) first part: pair-outer over 7 psum banks so
                # the matmuls chase the startup DMAs (term 1 needs only the
                # xh tiles; term 2 the xl tiles; term 3 the lo weights).
                pns = [c0_tile(ii) for ii in range(7)]

                def p1_term(j, wc, start=False, stop=False):
                    for ii in range(7):
                        nc.tensor.matmul(
                            pns[ii],
                            xhs[j][:, :, ii * 128:(ii + 1) * 128],
                            wc[:, j], start=start, stop=stop,
                            perf_mode=DR)
                # wl term lags one pair so the first matmuls need only
                # wh0[0] + xh[0] (wch0l is still in flight then)
                for j in range(NPAIR):
                    p1_term(j, wch0h, start=(j == 0))
                    if j >= 1:
                        p1_term(j - 1, wch0l)
                p1_term(NPAIR - 1, wch0l, stop=True)
                for ii in range(7):
                    rope_evac(pns[ii], ii, "v", 0)

                def load_wch(c, at_ms=None):
                    """Prefetch chunk c's weight pair one chunk ahead.
                    tile_wait_until parks the issue at roughly the right
                    sim time so the scheduler neither hoists it into the
                    startup queue (starving the critical x loads) nor
                    lets its WAR-wait seize SP.SEQ."""
                    with tc.tile_wait_until(at_ms, enable=at_ms is not None):
                        wh = paw.tile([128, NPAIR, 2, 512], FP8,
                                      name="wchh", tag="wchh")
                        nc.sync.dma_start(out=wh, in_=wq8h[c])
                        wl = paw.tile([128, NPAIR, 2, 512], FP8,
                                      name="wchl", tag="wchl")
                        nc.sync.dma_start(out=wl, in_=wq8l[c])
                    return wh, wl

                wnext = load_wch(1)
                # chunk 0 rest: data resident, i-outer so the RoPE
                # evacuations drain while the matmuls continue.
                for i in range(7, LT):
                    pnat = c0_tile(i)
                    qkv_mms_hi(pnat, i)
                    rope_evac(pnat, i, "v", 0)

                # chunks 1-4: i-outer, pair-inner (weights triple-buffered),
                # cycling through the 6 psA6 banks.
                for c in range(1, NCH - 1):
                    kind, grp = _chunk_kind(c)
                    wh, wl = wnext
                    wnext = load_wch(c + 1, at_ms=0.053 + (c - 1) * 0.047)
                    for i in range(LT):
                        pnat = pn_tile(i % 6)
                        qkv_mms(pnat, i, wh, wl)
                        rope_evac(pnat, i, kind, grp)
                        if c == 1 and i >= 8:
                            v03_fix(2 * (i - 8), 0)
                            v03_fix(2 * (i - 8) + 1, 1)
                    if c == 2:
                        # rotated q/k for heads 0-3 are now staged in
                        # DRAM: issue their transposed reloads early so
                        # they drain during the rest of phase A.
                        for h in range(4):
                            qt = pqk03.tile([128, L], FP16, name="qt03",
                                            tag=f"qt{h}", bufs=1)
                            nc.sync.dma_start_transpose(
                                out=qt,
                                in_=qrot[:, h * 128:(h + 1) * 128])
                            kt = pqk03.tile([128, L], FP16, name="kt03",
                                            tag=f"kt{h}", bufs=1)
                            nc.sync.dma_start_transpose(
                                out=kt,
                                in_=krot[:, h * 128:(h + 1) * 128])
                            qkts03.append((qt, kt))

            # RoPE pools (cos/sin, temps, psA8) close here; open the
            # attention pools that must outlive phase A, then emit chunk 5
            # (v47, no RoPE) with (heads 0-3, qc 0) attention interleaved.
            es_rope.close()
            pba = es.enter_context(
                tc.tile_pool(name="pBa", bufs=9, side="right"))
            pbr = es.enter_context(
                tc.tile_pool(name="pBr", bufs=2, side="right"))
            pyts = es.enter_context(
                tc.tile_pool(name="pYts", bufs=1, side="right"))
            pss = es.enter_context(
                tc.tile_pool(name="psS", bufs=scb, space="PSUM"))
            psy = es.enter_context(
                tc.tile_pool(name="psY", bufs=2, space="PSUM"))
            psd = es.enter_context(
                tc.tile_pool(name="psD", bufs=1, space="PSUM"))
            y8ts[0] = (pyts.tile([128, 4, L], FP8, name="y03h", tag="y03h"),
                       pyts.tile([128, 4, L], FP8, name="y03l", tag="y03l"))
            bpools = (pss, pba, psy, psd, pbr)
            pending = None
            c = NCH - 1
            kind, grp = _chunk_kind(c)
            wh, wl = wnext
            # 8 attention blocks hide under the v47 chunk: their exp lands
            # in the Act engine's phase-A idle window instead of the
            # Act-bound tail stretch
            inject = {0: (0, 0), 2: (0, 1), 4: (0, 2), 6: (0, 3),
                      8: (1, 0), 10: (1, 1), 12: (1, 2), 14: (1, 3)}
            for i in range(LT):
                pnat = pv_tile(i % 2)
                qkv_mms(pnat, i, wh, wl)
                rope_evac(pnat, i, kind, grp)
                if i in inject:
                    qci, hi = inject[i]
                    fin = attn_head_qc(bpools, hi, qci, dpool=av_tile)
                    if pending is not None:
                        pending()
                    pending = fin

        # ---------------- Phase B rest + Phase C ----------------
        pqkb = es.enter_context(
            tc.tile_pool(name="pQK47", bufs=1, side="right"))
        pcw = es.enter_context(tc.tile_pool(name="pCw", bufs=1))
        pco = es.enter_context(tc.tile_pool(name="pCo", bufs=4))
        for h in range(4, NH):
            qt = pqkb.tile([128, L], FP16, name="qt47", tag=f"qt{h}")
            nc.sync.dma_start_transpose(
                out=qt, in_=qrot[:, h * 128:(h + 1) * 128])
            kt = pqkb.tile([128, L], FP16, name="kt47", tag=f"kt{h}")
            nc.sync.dma_start_transpose(
                out=kt, in_=krot[:, h * 128:(h + 1) * 128])
            qkts47.append((qt, kt))
        y8ts[1] = (pqkb.tile([128, 4, L], FP8, name="y47h", tag="y47h"),
                   pqkb.tile([128, 4, L], FP8, name="y47l", tag="y47l"))
        for m in range(4):
            woh = pcw.tile([128, 2, D], FP8, name=f"woh{m}", tag=f"woh{m}")
            nc.sync.dma_start(out=woh, in_=wo8h[m])
            wol = pcw.tile([128, 2, D], FP8, name=f"wol{m}", tag=f"wol{m}")
            nc.sync.dma_start(out=wol, in_=wo8l[m])
            wo8ts.append((woh, wol))

        # remaining blocks: lead with qc2 heads 0-1 (their q/k are
        # resident) so each qc0 head-4-7 block waits at most one
        # in-flight transposed reload.
        seq = [(2, 0), (2, 1), (0, 4), (0, 5), (0, 6), (0, 7),
               (1, 4), (1, 5), (1, 6), (1, 7),
               (2, 2), (2, 3), (2, 4), (2, 5), (2, 6), (2, 7)]
        seq += [(3, h) for h in range(NH)]

        # phase C borrows the psAV banks (freed after chunk 5); pools on
        # the PSUM side must close LIFO so psAV simply stays open.
        def _ymov(m, hilo, qc):
            """Moving operand for out-proj pair m: heads (2m, 2m+1)."""
            t = y8ts[m // 2][hilo]
            r = (2 * m) % 4
            return t[:, r:r + 2, qc * 512:(qc + 1) * 512]

        def c_tile(qc, e, psum=None):
            op = psum() if psum is not None else av_tile()
            se = slice(e * 128, (e + 1) * 128)
            for m in range(4):
                nc.tensor.matmul(op, wo8ts[m][0][:, :, se], _ymov(m, 0, qc),
                                 start=(m == 0), stop=False, perf_mode=DR)
            for m in range(4):
                nc.tensor.matmul(op, wo8ts[m][0][:, :, se], _ymov(m, 1, qc),
                                 start=False, stop=False, perf_mode=DR)
            for m in range(4):
                nc.tensor.matmul(op, wo8ts[m][1][:, :, se], _ymov(m, 0, qc),
                                 start=False, stop=(m == 3), perf_mode=DR)
            ot = pco.tile([128, 512], FP16, name="ot", tag="ot")
            # DVE, not Act: the tail stretch is Act-bound (exp)
            nc.vector.tensor_scalar_mul(ot, op, ODESC)
            nc.sync.dma_start(
                out=outT[e * 128:(e + 1) * 128,
                         qc * 512:(qc + 1) * 512],
                in_=ot)

        cwork = []
        quota = [0]
        since_refill = [0]

        def filler():
            if cwork and quota[0] > 0:
                quota[0] -= 1
                cwork.pop(0)()

        # finales flushed during phase A: qc0 h0-3 and qc1 h0-2 (qc1 h3's
        # is the pending one entering the loop)
        emitted_fin = {0: 4, 1: 3, 2: 0, 3: 0}
        for k, (qc, h) in enumerate(seq):
            since_refill[0] += 1
            bl = max(1, 10 - since_refill[0])
            quota[0] = (len(cwork) + bl - 1) // bl if cwork else 0
            last = k == len(seq) - 1
            fin = attn_head_qc(bpools, h, qc, pe_denom=last,
                               dpool=av_tile, filler=filler)
            if pending is not None:
                pending()
                fq = seq[k - 1] if k > 0 else (1, 3)
                emitted_fin[fq[0]] += 1
                if emitted_fin[fq[0]] == NH and fq[0] < QC - 1:
                    cwork.extend(
                        (lambda qq, ee: lambda: c_tile(qq, ee))(
                            fq[0], e) for e in range(DT))
                    since_refill[0] = 0
            if last:
                # all-PE denominator: the finale is cheap, flush it
                # now so phase C can be emitted last
                fin()
                pending = None
            else:
                pending = fin
        quota[0] = len(cwork)
        while cwork:
            filler()
        # tail: attention is done, so the score psum banks are free —
        # rotate over 5 banks (2 psAV + 3 psS) to hide the evac latency
        def tail_psum(e):
            if e % 5 < 2:
                return av_tile
            return lambda: pss.tile([128, 512], F32, name="sc", tag="sc")
        for e in range(DT):
            c_tile(QC - 1, e, psum=tail_psum(e))
    nc.compile()
    return nc


_NC_CACHE = None


def _get_program():
    global _NC_CACHE
    if _NC_CACHE is None:
        _NC_CACHE = build_program()
    return _NC_CACHE


def _split8(a):
    """Hi/lo e4m3 split of an (already scaled) fp32 array."""
    h = a.astype(NP8)
    l = (a - h.astype(np.float32)).astype(NP8)
    return h, l


def _host_inputs(x, w_qkv, w_o):
    inv = 1.0 / (ROPE_BASE ** (np.arange(0, HD, 2, dtype=np.float64) / HD))
    ang = np.arange(L, dtype=np.float64)[:, None] * inv[None, :]
    chalf = np.tile(np.cos(ang), (1, 4)).astype(F16)          # [L, 256]
    shalf = np.tile(np.sin(ang), (1, 4)).astype(F16)
    p = np.arange(128)[:, None]
    f = np.arange(128)[None, :]
    trimask = (p <= f).astype(F16)                             # [128, 128]

    # per-batch x splits: [NPAIR, 128, 2, L] fp8
    xs = []
    for b in range(B):
        xT = np.ascontiguousarray(x[b].T) * np.float32(S_X)
        h8, l8 = _split8(xT)
        xs.append((
            np.ascontiguousarray(
                h8.reshape(NPAIR, 2, 128, L).transpose(0, 2, 1, 3)),
            np.ascontiguousarray(
                l8.reshape(NPAIR, 2, 128, L).transpose(0, 2, 1, 3)),
        ))

    # per-group w_qkv splits: [NCH, 128, NPAIR, 2, 512] fp8
    ws = []
    wos = []
    for g in range(2):
        qr = w_qkv[g * DL:(g + 1) * DL]
        kr = w_qkv[D + g * DL:D + (g + 1) * DL]
        vr = w_qkv[2 * D + g * DL:2 * D + (g + 1) * DL]
        wqkvT = np.ascontiguousarray(
            np.concatenate([vr[:512], qr[:512], kr[:512],
                            qr[512:], kr[512:], vr[512:]], axis=0).T
        ) * np.float32(S_W)                                    # [D, 3DL]
        h8, l8 = _split8(wqkvT)

        def _wlay(a):
            return np.ascontiguousarray(
                a.reshape(NPAIR, 2, 128, NCH, 512).transpose(3, 2, 0, 1, 4))
        ws.append((_wlay(h8), _wlay(l8)))

        woT = np.ascontiguousarray(
            w_o[:, g * DL:(g + 1) * DL].T) * np.float32(S_WO)  # [DL, D]
        h8, l8 = _split8(woT)

        def _olay(a):
            return np.ascontiguousarray(
                a.reshape(4, 2, 128, D).transpose(0, 2, 1, 3))
        wos.append((_olay(h8), _olay(l8)))

    in_maps = []
    for c in range(8):
        b, g = c % 4, c // 4
        in_maps.append({
            "x8h": xs[b][0],
            "x8l": xs[b][1],
            "wq8h": ws[g][0],
            "wq8l": ws[g][1],
            "wo8h": wos[g][0],
            "wo8l": wos[g][1],
            "chalf": chalf,
            "shalf": shalf,
            "trimask": trimask,
        })
    return in_maps


def kernel(x, w_qkv, w_o, _trace=False):
    x = np.asarray(x, dtype=np.float32)
    w_qkv = np.asarray(w_qkv, dtype=np.float32)
    w_o = np.asarray(w_o, dtype=np.float32)
    nc = _get_program()
    in_maps = _host_inputs(x, w_qkv, w_o)
    res = run_bass_kernel_spmd(nc, in_maps, core_ids=list(range(8)),
                               trace=_trace)
    kernel.last_result = res
    parts = [r["outT"] for r in res.results]
    out = np.empty((B, L, D), dtype=np.float32)
    for b in range(B):
        out[b] = (parts[b].astype(np.float32) +
                  parts[b + 4].astype(np.float32)).T
    return out
